# revision 1
# baseline (speedup 1.0000x reference)
"""Trainium2 Bass kernel for a 2-layer GATv2 GNN (nn_ComponentGNN).

Sharding: nodes (and their in-edges, grouped by destination) are partitioned
across 8 NeuronCores; small weights are replicated; source-node features are
exchanged with AllGather between layers; per-graph pooled sums are AllReduced.

Self-contained: host-side prep (edge sort / padding / index packing) +
Bass program build + SPMD run + output unshard all happen inside kernel().
"""
import math
import os
import sys

import numpy as np

sys.path.insert(0, "/opt/trn_rl_repo")

NEG_SLOPE = 0.2
EPS = 1e-5
HEADS = 4
HID = 64
F_IN = 256
NCORES = 8
NQ = 4  # src-table quarters (dma_gather idx is int16)


# ---------------------------------------------------------------- config ----
class Cfg:
    def __init__(self, N, E, G, KQ):
        self.N, self.E, self.G = N, E, G
        assert N % NCORES == 0
        self.NS_REAL = N // NCORES
        self.NS = ((self.NS_REAL + 127) // 128) * 128
        self.NB = self.NS // 128           # dst blocks per core
        self.NTAB = self.NS * NCORES       # rows in full tables
        assert self.NTAB % NQ == 0
        self.QROWS = self.NTAB // NQ
        assert self.QROWS <= 32768
        # block groups (bound SBUF accumulator size)
        nbg = min(4, self.NB)
        base, rem = divmod(self.NB, nbg)
        self.BG = [base + (1 if i < rem else 0) for i in range(nbg)]
        self.BG_START = np.cumsum([0] + self.BG)
        self.KQ = KQ
        self.CAP = KQ * 128                # slots per (block, quarter)
        self.S = self.NB * NQ * self.CAP   # slots per core
        # gather-chunk sizes in blocks
        self.CB0 = min(5, min(self.BG))
        self.CB1 = min(2, min(self.BG))


def _table_row(nid, cfg):
    return (nid // cfg.NS_REAL) * cfg.NS + (nid % cfg.NS_REAL)


def host_prep(inputs, cfg):
    """Sort/pad edges, pack gather indices, fold BN; returns per-core arrays."""
    c = cfg
    edge_index = np.asarray(inputs["edge_index"])
    batch = np.asarray(inputs["batch"])
    src = edge_index[0].astype(np.int64)
    dst = edge_index[1].astype(np.int64)

    core = dst // c.NS_REAL
    dstl = dst % c.NS_REAL
    blk = dstl // 128
    srow = _table_row(src, c)
    q = srow // c.QROWS
    srcq = srow % c.QROWS
    bg = np.searchsorted(c.BG_START[1:], blk, side="right")

    ncell = NCORES * len(c.BG) * NQ * c.NB
    keys = ((core * len(c.BG) + bg) * NQ + q) * c.NB + blk
    cnt = np.bincount(keys, minlength=ncell)
    assert cnt.max() <= c.CAP, (cnt.max(), c.CAP)

    bgarr = np.array(c.BG)
    bg_base = np.concatenate([[0], np.cumsum(bgarr * NQ * c.CAP)])[:-1]
    slot_base = bg_base[bg] + q * bgarr[bg] * c.CAP + (blk - c.BG_START[bg]) * c.CAP

    order = np.lexsort((slot_base, core))
    so_core = core[order]
    so_base = slot_base[order]
    cell_key = so_core.astype(np.int64) * c.S + so_base
    chg = np.empty(len(cell_key), dtype=bool)
    chg[0] = True
    chg[1:] = cell_key[1:] != cell_key[:-1]
    seg_start = np.maximum.accumulate(np.where(chg, np.arange(len(cell_key)), 0))
    slot = so_base + (np.arange(len(cell_key)) - seg_start)

    srcq_idx = np.zeros((NCORES, c.S), np.int16)
    dst_idx = np.zeros((NCORES, c.S), np.int16)
    dstrel = np.full((NCORES, c.S), 999.0, np.float32)
    srcq_idx[so_core, slot] = srcq[order].astype(np.int16)
    dst_idx[so_core, slot] = dstl[order].astype(np.int16)
    dstrel[so_core, slot] = (dstl[order] - blk[order] * 128).astype(np.float32)

    def wrap16(a):  # [S] -> [128, S//16] int16 (idx j at [j%16, j//16], tiled x8)
        w = a.reshape(-1, 16).T  # [16, S/16]
        return np.tile(w, (8, 1)).copy()

    srcq_w = np.stack([wrap16(srcq_idx[i]) for i in range(NCORES)])
    dst_w = np.stack([wrap16(dst_idx[i]) for i in range(NCORES)])
    # dstrel e-major: [128, S//128], [p, t] = slot 128t+p
    dstrel_pm = dstrel.reshape(NCORES, c.S // 128, 128).transpose(0, 2, 1).copy()

    batchloc = np.full((NCORES, c.NS), 999.0, np.float32)
    for i in range(NCORES):
        batchloc[i, :c.NS_REAL] = batch[i * c.NS_REAL:(i + 1) * c.NS_REAL]
    batchloc_pm = batchloc.reshape(NCORES, c.NB, 128).transpose(0, 2, 1).copy()

    # x shards, transposed ([256, NS], zero-padded)
    x = np.asarray(inputs["x"], np.float32)
    xT = np.zeros((NCORES, F_IN, c.NS), np.float32)
    for i in range(NCORES):
        xT[i, :, :c.NS_REAL] = x[i * c.NS_REAL:(i + 1) * c.NS_REAL].T

    def fold(bias, g, be, m, v, head_mean):
        a = np.asarray(g, np.float64) / np.sqrt(np.asarray(v, np.float64) + EPS)
        B = a * (np.asarray(bias, np.float64) - np.asarray(m, np.float64)) + np.asarray(be, np.float64)
        A = a * (0.25 if head_mean else 1.0)
        return A.astype(np.float32), B.astype(np.float32)

    A0, B0 = fold(inputs["bias0"], inputs["g0"], inputs["be0"], inputs["m0"], inputs["v0"], False)
    A1, B1 = fold(inputs["bias1"], inputs["g1"], inputs["be1"], inputs["m1"], inputs["v1"], True)

    f32 = lambda k: np.ascontiguousarray(np.asarray(inputs[k], np.float32))
    rep = dict(
        W_in=f32("W_in"), b_in_c=f32("b_in").reshape(-1, 1),
        Wl0=f32("Wl0"), Wr0=f32("Wr0"),
        bl0_r=f32("bl0").reshape(1, -1), br0_r=f32("br0").reshape(1, -1),
        att0_r=f32("att0").reshape(1, -1),
        Wl1=f32("Wl1"), Wr1=f32("Wr1"),
        bl1_r=f32("bl1").reshape(1, -1), br1_r=f32("br1").reshape(1, -1),
        att1_r=f32("att1").reshape(1, -1),
        A0_r=A0.reshape(1, -1), B0_r=B0.reshape(1, -1),
        A1_r=A1.reshape(1, -1), B1_r=B1.reshape(1, -1),
        Wc1=f32("Wc1"), bc1_c=f32("bc1").reshape(-1, 1),
        Wc2=f32("Wc2"), bc2_c=f32("bc2").reshape(-1, 1),
        iota128_r=np.arange(128, dtype=np.float32).reshape(1, 128),
        iotaG_r=np.arange(cfg.G, dtype=np.float32).reshape(1, -1),
        ident=np.eye(128, dtype=np.float32),
    )
    per_core = [dict(
        xT=xT[i], srcq_w=srcq_w[i], dst_w=dst_w[i],
        dstrel=dstrel_pm[i], batchloc=batchloc_pm[i], **rep,
    ) for i in range(NCORES)]
    return per_core


def compute_kq(inputs, cfg_like=None):
    """KQ (uniform per-(block,quarter) ktile cap) from the actual edge data."""
    edge_index = np.asarray(inputs["edge_index"])
    N = int(np.asarray(inputs["x"]).shape[0])
    NS_REAL = N // NCORES
    NS = ((NS_REAL + 127) // 128) * 128
    NB = NS // 128
    QROWS = NS * NCORES // NQ
    src = edge_index[0].astype(np.int64)
    dst = edge_index[1].astype(np.int64)
    core = dst // NS_REAL
    dstl = dst % NS_REAL
    blk = dstl // 128
    srow = (src // NS_REAL) * NS + (src % NS_REAL)
    q = srow // QROWS
    keys = (core * NQ + q) * NB + blk
    cnt = np.bincount(keys, minlength=NCORES * NQ * NB)
    return int(math.ceil(cnt.max() / 128))


# ---------------------------------------------------------------- device ----
def build_program(cfg, reps=1):
    from contextlib import ExitStack
    import concourse.bass as bass
    import concourse.tile as tile
    from concourse import bacc, mybir

    c = cfg
    f32 = mybir.dt.float32
    i16 = mybir.dt.int16
    AF = mybir.ActivationFunctionType
    OP = mybir.AluOpType

    nc = bacc.Bacc("TRN2", target_bir_lowering=False, debug=False,
                   num_devices=NCORES)

    def din(name, shape, dt=f32):
        return nc.dram_tensor(name, list(shape), dt, kind="ExternalInput").ap()

    # --- inputs
    xT = din("xT", [F_IN, c.NS])
    srcq_w = din("srcq_w", [128, c.S // 16], i16)
    dst_w = din("dst_w", [128, c.S // 16], i16)
    dstrel_d = din("dstrel", [128, c.S // 128])
    batchloc_d = din("batchloc", [128, c.NB])
    W_in = din("W_in", [F_IN, HID]); b_in_c = din("b_in_c", [HID, 1])
    Wl0 = din("Wl0", [HID, HID]); Wr0 = din("Wr0", [HID, HID])
    bl0_r = din("bl0_r", [1, HID]); br0_r = din("br0_r", [1, HID])
    att0_r = din("att0_r", [1, HID])
    Wl1 = din("Wl1", [HID, 256]); Wr1 = din("Wr1", [HID, 256])
    bl1_r = din("bl1_r", [1, 256]); br1_r = din("br1_r", [1, 256])
    att1_r = din("att1_r", [1, 256])
    A0_r = din("A0_r", [1, HID]); B0_r = din("B0_r", [1, HID])
    A1_r = din("A1_r", [1, HID]); B1_r = din("B1_r", [1, HID])
    Wc1 = din("Wc1", [HID, 32]); bc1_c = din("bc1_c", [32, 1])
    Wc2 = din("Wc2", [32, 2]); bc2_c = din("bc2_c", [2, 1])
    iota128_r = din("iota128_r", [1, 128])
    iotaG_r = din("iotaG_r", [1, c.G])
    ident = din("ident", [128, 128])

    out_t = nc.dram_tensor("out_t", [2, c.G], f32, kind="ExternalOutput").ap()

    # --- internal DRAM
    def dram(name, shape, shared=False):
        return nc.dram_tensor(name, list(shape), f32, kind="Internal",
                              addr_space="Shared" if shared else "Local").ap()

    h0T_sh = dram("h0T_sh", [HID, c.NS])
    h0T_full = dram("h0T_full", [NCORES * HID, c.NS], shared=True)
    xl0_tab = dram("xl0_tab", [c.NTAB, HID])
    xr0_tab = dram("xr0_tab", [c.NS, HID])
    h1T_sh = dram("h1T_sh", [HID, c.NS])
    h1T_full = dram("h1T_full", [NCORES * HID, c.NS], shared=True)
    xl1_tab = dram("xl1_tab", [c.NTAB, 256])
    xr1_tab = dram("xr1_tab", [c.NS, 256])
    pool_in = dram("pool_in", [c.G, HID + 1])
    pool_out = dram("pool_out", [c.G, HID + 1], shared=True)

    RG = [list(range(NCORES))]

    def bc(ap, dims):
        """Manual broadcast: dims = list of (src_axis_or_None, size)."""
        pat = []
        for ax, size in dims:
            if ax is None:
                pat.append([0, size])
            else:
                st, sz = ap.ap[ax]
                assert sz == size, (ap.ap, ax, size)
                pat.append([st, size])
        return bass.AP(tensor=ap.tensor, offset=ap.offset, ap=pat)

    with tile.TileContext(nc) as tc, ExitStack() as ctx:
        singles = ctx.enter_context(tc.tile_pool(name="singles", bufs=1))

        _cn = [0]

        def load_const(ap_d, shape, dt=f32):
            _cn[0] += 1
            t = singles.tile(list(shape), dt, name=f"const{_cn[0]}")
            nc.sync.dma_start(out=t[:], in_=ap_d)
            return t

        def load_row128(ap_d, width):
            """Replicate a [1, width] DRAM row to a [128, width] SBUF tile."""
            _cn[0] += 1
            t = singles.tile([128, width], f32, name=f"const{_cn[0]}")
            src = bass.AP(tensor=ap_d.tensor, offset=ap_d.offset,
                          ap=[[0, 128], [1, width]])
            nc.sync.dma_start(out=t[:], in_=src)
            return t

        # W_in is [256, 64]; needs two [128, 64] SBUF tiles (K halves).
        W_in_h = []
        for kh in range(F_IN // 128):
            t = singles.tile([128, HID], f32, name=f"Win{kh}")
            nc.sync.dma_start(out=t[:], in_=W_in[kh * 128:(kh + 1) * 128, :])
            W_in_h.append(t)
        b_in_sb = load_const(b_in_c, [HID, 1])
        Wl0_sb = load_const(Wl0, [HID, HID]); Wr0_sb = load_const(Wr0, [HID, HID])
        bl0_sb = load_row128(bl0_r, HID); br0_sb = load_row128(br0_r, HID)
        att0_sb = load_row128(att0_r, HID)
        Wl1_sb = load_const(Wl1, [HID, 256]); Wr1_sb = load_const(Wr1, [HID, 256])
        bl1_sb = load_row128(bl1_r, 256); br1_sb = load_row128(br1_r, 256)
        att1_sb = load_row128(att1_r, 256)
        A0_sb = load_row128(A0_r, HID); B0_sb = load_row128(B0_r, HID)
        A1_sb = load_row128(A1_r, HID); B1_sb = load_row128(B1_r, HID)
        Wc1_sb = load_const(Wc1, [HID, 32]); bc1_sb = load_const(bc1_c, [32, 1])
        Wc2_sb = load_const(Wc2, [32, 2]); bc2_sb = load_const(bc2_c, [2, 1])
        io128_sb = load_row128(iota128_r, 128)
        ioG_sb = load_row128(iotaG_r, c.G)
        ident_sb = load_const(ident, [128, 128])
        batchloc_sb = load_const(batchloc_d, [128, c.NB])

        # ---------------- P1: h0T shard = relu(W_in^T @ x + b) --------------
        for _rep in range(reps):
         with tc.tile_pool(name="p1", bufs=3) as p1, \
              tc.tile_pool(name="p1ps", bufs=2, space="PSUM") as p1ps:
             n0 = 0
             while n0 < c.NS:
                 nw = min(512, c.NS - n0)
                 ps = p1ps.tile([HID, 512], f32)
                 for kh in range(F_IN // 128):
                     xt_t = p1.tile([128, 512], f32, name="xt")
                     nc.sync.dma_start(out=xt_t[:, :nw],
                                       in_=xT[kh * 128:(kh + 1) * 128, n0:n0 + nw])
                     nc.tensor.matmul(ps[:, :nw], W_in_h[kh][:], xt_t[:, :nw],
                                      start=(kh == 0), stop=(kh == F_IN // 128 - 1))
                 h0_t = p1.tile([HID, 512], f32, name="h0t")
                 nc.scalar.activation(h0_t[:, :nw], ps[:, :nw], AF.Relu,
                                      bias=b_in_sb[:], scale=1.0)
                 nc.sync.dma_start(out=h0T_sh[:, n0:n0 + nw], in_=h0_t[:, :nw])
                 n0 += nw

         # ---------------- AG h0T -> h0T_full --------------------------------
         nc.gpsimd.collective_compute(
             "AllGather", mybir.AluOpType.bypass, ins=[h0T_sh],
             outs=[h0T_full], replica_groups=RG)

         # ---------------- table builds --------------------------------------
         def build_table(tab_ap, n_rows_slabs, W_sb, b_sb, width, hT_src):
             """tab[r,:] = hT_src_slab^T @ W + b, n-major rows."""
             with tc.tile_pool(name="tb", bufs=3) as tb, \
                  tc.tile_pool(name="tbps", bufs=2, space="PSUM") as tbps:
                 psum_cols = min(width, 512 // 1)  # psum free f32 max 512
                 ntile_per_ps = max(1, 512 // width)
                 for slab in range(n_rows_slabs):
                     n0 = 0
                     while n0 < c.NS:
                         nt = min(ntile_per_ps, (c.NS - n0) // 128)
                         lh = tb.tile([HID, ntile_per_ps * 128], f32, name="lh")
                         nc.sync.dma_start(
                             out=lh[:, :nt * 128],
                             in_=hT_src[slab * HID:(slab + 1) * HID, n0:n0 + nt * 128])
                         ps = tbps.tile([128, ntile_per_ps, width], f32)
                         for j in range(nt):
                             nc.tensor.matmul(
                                 ps[:, j, :], lh[:, j * 128:(j + 1) * 128], W_sb[:],
                                 start=True, stop=True)
                         ot = tb.tile([128, ntile_per_ps, width], f32, name="ot")
                         nc.vector.tensor_tensor(
                             out=ot[:, :nt, :], in0=ps[:, :nt, :],
                             in1=bc(b_sb[:], [(0, 128), (None, nt), (1, width)]),
                             op=OP.add)
                         dst_rows = tab_ap[slab * c.NS + n0: slab * c.NS + n0 + nt * 128, :]
                         nc.sync.dma_start(
                             out=dst_rows.rearrange("(t p) w -> p t w", p=128),
                             in_=ot[:, :nt, :])
                         n0 += nt * 128

         build_table(xl0_tab, NCORES, Wl0_sb, bl0_sb, HID, h0T_full)
         build_table(xr0_tab, 1, Wr0_sb, br0_sb, HID, h0T_sh)

         # ---------------- edge phase ----------------------------------------
         def edge_phase(xl_tab, xr_tab, att_sb, A_sb, B_sb, ch, head_mean, CB,
                        post_block):
             """GATv2 message passing; post_block(b_global, h_tile[128, HID])
             is called with the finished per-block node rows."""
             W = HEADS * ch
             WP = W + HEADS
             with tc.tile_pool(name="eg", bufs=2) as eg, \
                  tc.tile_pool(name="acc_p", bufs=1) as acc_p, \
                  tc.tile_pool(name="egps", bufs=4, space="PSUM") as egps, \
                  tc.tile_pool(name="nrm", bufs=2) as nrm:
                 acc = acc_p.tile([128, max(c.BG), WP], f32)
                 for bgi, gsz in enumerate(c.BG):
                     bg0 = int(c.BG_START[bgi])
                     bg_slot0 = c.NB * NQ * c.CAP * 0  # recomputed below
                     bgarr = np.array(c.BG)
                     bg_base = int(np.concatenate([[0], np.cumsum(bgarr * NQ * c.CAP)])[bgi])
                     for q in range(NQ):
                         j0 = 0
                         while j0 < gsz:
                             cb = min(CB, gsz - j0)
                             T = cb * c.KQ           # ktiles in chunk
                             s0 = bg_base + q * gsz * c.CAP + j0 * c.CAP
                             # idx/meta loads
                             sq_t = eg.tile([128, CB * c.KQ * 8], i16, name="sq")
                             nc.sync.dma_start(out=sq_t[:, :T * 8],
                                               in_=srcq_w[:, s0 // 16:(s0 + T * 128) // 16])
                             dq_t = eg.tile([128, CB * c.KQ * 8], i16, name="dq")
                             nc.sync.dma_start(out=dq_t[:, :T * 8],
                                               in_=dst_w[:, s0 // 16:(s0 + T * 128) // 16])
                             dr_t = eg.tile([128, CB * c.KQ], f32, name="dr")
                             nc.sync.dma_start(out=dr_t[:, :T],
                                               in_=dstrel_d[:, s0 // 128:s0 // 128 + T])
                             # gathers
                             xl_t = eg.tile([128, CB * c.KQ, W], f32, name="xl")
                             nc.gpsimd.dma_gather(
                                 xl_t[:, :T, :],
                                 xl_tab[q * c.QROWS:(q + 1) * c.QROWS, :],
                                 sq_t[:, :T * 8], T * 128, T * 128, W,
                                 single_packet=False)
                             xr_t = eg.tile([128, CB * c.KQ, W], f32, name="xr")
                             nc.gpsimd.dma_gather(
                                 xr_t[:, :T, :], xr_tab[:, :],
                                 dq_t[:, :T * 8], T * 128, T * 128, W,
                                 single_packet=False)
                             # onehot
                             oh_t = eg.tile([128, CB * c.KQ, 128], f32, name="oh")
                             nc.vector.tensor_tensor(
                                 out=oh_t[:, :T, :],
                                 in0=bc(dr_t[:, :T], [(0, 128), (1, T), (None, 128)]),
                                 in1=bc(io128_sb[:], [(0, 128), (None, T), (1, 128)]),
                                 op=OP.is_equal)
                             # z = lrelu(xl + xr)
                             z_t = eg.tile([128, CB * c.KQ, W], f32, name="z")
                             nc.vector.tensor_tensor(out=z_t[:, :T, :], in0=xl_t[:, :T, :],
                                                     in1=xr_t[:, :T, :], op=OP.add)
                             # lrelu: reuse xr_t as 0.2*z scratch (xr is dead)
                             nc.vector.tensor_scalar_mul(xr_t[:, :T, :], z_t[:, :T, :],
                                                         NEG_SLOPE)
                             nc.vector.tensor_tensor(out=z_t[:, :T, :], in0=z_t[:, :T, :],
                                                     in1=xr_t[:, :T, :], op=OP.max)
                             # logits = sum_c z*att per head (att-mul in place)
                             nc.vector.tensor_tensor(
                                 out=z_t[:, :T, :], in0=z_t[:, :T, :],
                                 in1=bc(att_sb[:], [(0, 128), (None, T), (1, W)]),
                                 op=OP.mult)
                             ywp_t = eg.tile([128, CB * c.KQ, WP], f32, name="ywp")
                             lg_t = eg.tile([128, CB * c.KQ, HEADS], f32, name="lg")
                             nc.vector.tensor_reduce(
                                 out=lg_t[:, :T, :],
                                 in_=z_t[:, :T, :].rearrange("p t (h c) -> p t h c", h=HEADS),
                                 axis=mybir.AxisListType.X, op=OP.add)
                             nc.scalar.activation(
                                 ywp_t[:, :T, W:WP], lg_t[:, :T, :], AF.Exp)
                             # Yw = xl * w
                             nc.vector.tensor_tensor(
                                 out=ywp_t[:, :T, 0:W].rearrange("p t (h c) -> p t h c", h=HEADS),
                                 in0=xl_t[:, :T, :].rearrange("p t (h c) -> p t h c", h=HEADS),
                                 in1=bc(ywp_t[:, :T, W:WP],
                                        [(0, 128), (1, T), (2, HEADS), (None, ch)]),
                                 op=OP.mult)
                             # per-block reduce
                             for j in range(cb):
                                 ps = egps.tile([128, WP], f32, name="rps")
                                 for kt in range(c.KQ):
                                     ix = j * c.KQ + kt
                                     nc.tensor.matmul(ps[:], oh_t[:, ix, :],
                                                      ywp_t[:, ix, :],
                                                      start=(kt == 0),
                                                      stop=(kt == c.KQ - 1))
                                 jb = j0 + j
                                 if q == 0:
                                     nc.vector.tensor_copy(out=acc[:, jb, :], in_=ps[:])
                                 else:
                                     nc.vector.tensor_tensor(out=acc[:, jb, :],
                                                             in0=acc[:, jb, :],
                                                             in1=ps[:], op=OP.add)
                             j0 += cb
                     # normalize this block group
                     for j in range(gsz):
                         b_glob = bg0 + j
                         den_t = nrm.tile([128, HEADS], f32, name="den")
                         nc.vector.tensor_scalar_max(den_t[:], acc[:, j, W:WP], 1e-30)
                         rec_t = nrm.tile([128, HEADS], f32, name="rec")
                         nc.vector.reciprocal(rec_t[:], den_t[:])
                         v_t = nrm.tile([128, W], f32, name="v")
                         nc.vector.tensor_tensor(
                             out=v_t[:].rearrange("p (h c) -> p h c", h=HEADS),
                             in0=acc[:, j, 0:W].rearrange("p (h c) -> p h c", h=HEADS),
                             in1=bc(rec_t[:], [(0, 128), (1, HEADS), (None, ch)]),
                             op=OP.mult)
                         if head_mean:
                             hsum = nrm.tile([128, HID], f32, name="hsum")
                             vap = v_t[:]  # [128, 4*64], strides: h-major
                             nc.vector.tensor_reduce(
                                 out=hsum[:],
                                 in_=bass.AP(tensor=vap.tensor, offset=vap.offset,
                                             ap=[vap.ap[0], [1, HID], [HID, HEADS]]),
                                 axis=mybir.AxisListType.X, op=OP.add)
                             vv = hsum
                         else:
                             vv = v_t
                         h_t = nrm.tile([128, HID], f32, name="hfin")
                         nc.vector.tensor_tensor(
                             out=h_t[:], in0=vv[:],
                             in1=bc(A_sb[:], [(0, 128), (1, HID)]), op=OP.mult)
                         nc.vector.tensor_tensor(
                             out=h_t[:], in0=h_t[:],
                             in1=bc(B_sb[:], [(0, 128), (1, HID)]), op=OP.add)
                         nc.vector.tensor_scalar_max(h_t[:], h_t[:], 0.0)
                         post_block(b_glob, h_t)

         # ---- L0 post-block: transpose to h1T shard
         with tc.tile_pool(name="l0post", bufs=2) as l0post, \
              tc.tile_pool(name="l0ps", bufs=2, space="PSUM") as l0ps:
             h1T_buf = {"tile": None, "b0": 0, "n": 0}

             def l0_post(b_glob, h_t):
                 ps = l0ps.tile([HID, 128], f32, name="tps")
                 nc.tensor.transpose(ps[:], h_t[:], ident_sb[:])
                 if h1T_buf["tile"] is None or h1T_buf["n"] == 4 or \
                    b_glob != h1T_buf["b0"] + h1T_buf["n"]:
                     if h1T_buf["tile"] is not None:
                         nn = h1T_buf["n"]
                         nc.sync.dma_start(
                             out=h1T_sh[:, h1T_buf["b0"] * 128:(h1T_buf["b0"] + nn) * 128],
                             in_=h1T_buf["tile"][:, :nn * 128])
                     h1T_buf["tile"] = l0post.tile([HID, 512], f32, name="h1Tb")
                     h1T_buf["b0"] = b_glob
                     h1T_buf["n"] = 0
                 nc.vector.tensor_copy(
                     out=h1T_buf["tile"][:, h1T_buf["n"] * 128:(h1T_buf["n"] + 1) * 128],
                     in_=ps[:])
                 h1T_buf["n"] += 1

             edge_phase(xl0_tab, xr0_tab, att0_sb, A0_sb, B0_sb, 16, False,
                        c.CB0, l0_post)
             if h1T_buf["tile"] is not None:
                 nn = h1T_buf["n"]
                 nc.sync.dma_start(
                     out=h1T_sh[:, h1T_buf["b0"] * 128:(h1T_buf["b0"] + nn) * 128],
                     in_=h1T_buf["tile"][:, :nn * 128])

         # ---------------- AG h1T, build L1 tables ---------------------------
         nc.gpsimd.collective_compute(
             "AllGather", mybir.AluOpType.bypass, ins=[h1T_sh],
             outs=[h1T_full], replica_groups=RG)
         build_table(xl1_tab, NCORES, Wl1_sb, bl1_sb, 256, h1T_full)
         build_table(xr1_tab, 1, Wr1_sb, br1_sb, 256, h1T_sh)

         # ---------------- L1 edge phase + pooling ---------------------------
         with tc.tile_pool(name="l1post", bufs=2) as l1post, \
              tc.tile_pool(name="poolps", bufs=1, space="PSUM") as poolps:
             pooled_ps = poolps.tile([c.G, HID + 1], f32)
             blk_count = {"n": 0}

             def l1_post(b_glob, h_t):
                 he = l1post.tile([128, HID + 1], f32, name="h2e")
                 nc.vector.tensor_copy(out=he[:, :HID], in_=h_t[:])
                 nc.vector.memset(he[:, HID:HID + 1], 1.0)
                 og = l1post.tile([128, c.G], f32, name="og")
                 nc.vector.tensor_tensor(
                     out=og[:],
                     in0=bc(batchloc_sb[:, b_glob:b_glob + 1], [(0, 128), (None, c.G)]),
                     in1=bc(ioG_sb[:], [(0, 128), (1, c.G)]),
                     op=OP.is_equal)
                 nc.tensor.matmul(pooled_ps[:], og[:], he[:],
                                  start=(blk_count["n"] == 0),
                                  stop=(blk_count["n"] == c.NB - 1))
                 blk_count["n"] += 1

             edge_phase(xl1_tab, xr1_tab, att1_sb, A1_sb, B1_sb, HID, True,
                        c.CB1, l1_post)

             # pooled partial -> AllReduce
             pool_sb = l1post.tile([c.G, HID + 1], f32, name="poolsb")
             nc.vector.tensor_copy(out=pool_sb[:], in_=pooled_ps[:])
             nc.sync.dma_start(out=pool_in[:, :], in_=pool_sb[:])

         nc.gpsimd.collective_compute(
             "AllReduce", mybir.AluOpType.add, ins=[pool_in],
             outs=[pool_out], replica_groups=RG)

         # ---------------- classifier ---------------------------------------
         with tc.tile_pool(name="cls", bufs=1) as cls, \
              tc.tile_pool(name="clsps", bufs=2, space="PSUM") as clsps:
             pall = cls.tile([c.G, HID + 1], f32)
             nc.sync.dma_start(out=pall[:], in_=pool_out[:, :])
             cnt = cls.tile([c.G, 1], f32)
             nc.vector.tensor_scalar_max(cnt[:], pall[:, HID:HID + 1], 1.0)
             rcnt = cls.tile([c.G, 1], f32)
             nc.vector.reciprocal(rcnt[:], cnt[:])
             pm = cls.tile([c.G, HID], f32)
             nc.vector.tensor_scalar_mul(pm[:], pall[:, :HID], rcnt[:])
             # transpose pooled -> [HID, G]
             pmT_ps = clsps.tile([HID, c.G], f32)
             nc.tensor.transpose(pmT_ps[:], pm[:], ident_sb[:c.G, :c.G])
             pmT = cls.tile([HID, c.G], f32)
             nc.vector.tensor_copy(out=pmT[:], in_=pmT_ps[:])
             z1_ps = clsps.tile([32, c.G], f32)
             nc.tensor.matmul(z1_ps[:], Wc1_sb[:], pmT[:], start=True, stop=True)
             z1 = cls.tile([32, c.G], f32)
             nc.scalar.activation(z1[:], z1_ps[:], AF.Relu, bias=bc1_sb[:])
             o_ps = clsps.tile([2, c.G], f32)
             nc.tensor.matmul(o_ps[:], Wc2_sb[:], z1[:], start=True, stop=True)
             o_sb = cls.tile([2, c.G], f32)
             nc.scalar.activation(o_sb[:], o_ps[:], AF.Identity, bias=bc2_sb[:])
             nc.sync.dma_start(out=out_t[:, :], in_=o_sb[:])

    nc.compile()
    return nc


# ---------------------------------------------------------------- driver ----
_BUILT = {}
LAST_RESULTS = None


def _get_program(cfg):
    key = (cfg.N, cfg.E, cfg.G, cfg.KQ)
    if key not in _BUILT:
        _BUILT[key] = build_program(cfg)
    return _BUILT[key]


def kernel(**inputs):
    from concourse import bass_utils

    x = np.asarray(inputs["x"])
    edge_index = np.asarray(inputs["edge_index"])
    batch = np.asarray(inputs["batch"])
    N = x.shape[0]
    E = edge_index.shape[1]
    G = 64
    KQ = compute_kq(inputs)
    cfg = Cfg(N, E, G, KQ)
    per_core = host_prep(inputs, cfg)
    nc = _get_program(cfg)
    in_maps = [{k: np.ascontiguousarray(v) for k, v in m.items()} for m in per_core]
    res = bass_utils.run_bass_kernel_spmd(nc, in_maps, core_ids=list(range(NCORES)))
    global LAST_RESULTS
    LAST_RESULTS = res
    out = res.results[0]["out_t"]  # [2, G]
    return np.ascontiguousarray(out.T.astype(np.float32))

